# revision 31
# baseline (speedup 1.0000x reference)
"""Trainium2 Bass kernel for equivariant multihead attention.

Math (per batch b, query point i, coset s1, channel c):
    logit[j,s2] = sum_g pairwise_g[b,i,j,s1,s2,g]*w_g[c,g]
                  + w_y[c,0]*y[b,j,s2,c] + w_y[c,1]*y[b,i,s1,c] + b_g[c] + b_y[c]
    att = exp(logit)*mask[b,j,s2];  att /= sum_{j,s2} att
    out = (y[b,i,s1,c] + sum_{j,s2} att*y[b,j,s2,c]) * mask[b,i,s1]  @ w_lin.T

The query-side term and the biases are constant over the key dims (j,s2) and
cancel in the normalization, so they are dropped.  The key-side factor
exp(w_y[c,0]*y)*mask is folded INTO the exponent: with
    L[(s2,c), j] = w_y0[c]*y[b,j,s2,c] + log(mask[b,j,s2])   (-1e30 if masked)
the unnormalized attention is E' = exp(sum_g pg*w_g + L) directly, and
    den[s1,c] = sum_{j,s2} E'               (free-dim accumulate during exp)
    num[s1,c] = sum_{j,s2} E' * y[b,j,s2,c] (one fused multiply-reduce)

TWO query points (a "pair") are packed per instruction: output partition is
(i2, s1, c) = 2*8*8 = 128 and the free dim is (h, t, j) = 2*4*128 = 1024
(s2 = 2t+h lives entirely in the free dim, so the per-partition accumulate
gives complete per-(i,s1,c) sums).  Device work per pair is FOUR instructions:
  * two 120x128 @ 120x512 matmuls (contraction: 112 pairwise-g rows for both
    i2 + 8 L-rows added via an indicator block in the fixed stationary),
  * exp on the Act engine (128x1024) with accum_out -> den column,
  * one scalar_tensor_tensor against a y-table with accum_out -> num column.
The host pre-transposes pairwise_g into the rhs stream (bf16, halves DMA) and
finishes with the residual add, query mask, and the c_in->c_out linear.

Sharding: query dim i is split 8 ways (16 i x 4 b = 32 pairs per core).
"""

import numpy as np

import concourse.bacc as bacc
import concourse.tile as tile
from concourse import mybir
from concourse.bass_utils import run_bass_kernel_spmd

B, N, S, CIN, COUT, GDIM = 4, 128, 8, 8, 8, 7
NCORES = 8
ISHARD = N // NCORES          # 16 query points per core
NBLK = B * ISHARD             # 64 (b,i) blocks per core
NPAIR = NBLK // 2             # 32 block pairs per core
NT = 4                        # s2-pair tiles
PW = 2 * NT * N               # 1024 rhs columns per pair: (h, t, j)
CROWS = 120                   # contraction rows: 112 pairwise-g + 8 L

# blob column layout (bf16): [lhsT | YT tables | rhs stream].  The key
# axis (s2, j) is COMPACTED per batch to the mask-valid columns (masked
# keys contribute exactly 0), padded to a static width padw; _host_prep
# computes the layout from the actual mask and updates _LAYOUT.
LHS0 = 0
YT0 = 128
STR0 = YT0 + B * PW           # 4224 (dense fallback)
TOTW = STR0 + NPAIR * PW      # 36992 (dense fallback)
_LAYOUT = {"padw": PW, "str0": STR0, "totw": TOTW}

# pairs covered by each stream DMA (earlier ones smaller for pipeline ramp)
SUPER_PAIRS = (2, 4, 5, 6, 7, 8)

F32 = mybir.dt.float32
BF16 = mybir.dt.bfloat16
NPBF16 = mybir.dt.np(mybir.dt.bfloat16)

_PROGRAM_CACHE = {}


def _build_program(nblk=NBLK, loop_reps=1, mode="full"):
    """loop_reps>1 wraps the main loop in a hardware For_i that re-runs the
    full pass (including the input DMAs) on the same data -- used only for
    timing: wall(loop_reps=R) - wall(loop_reps=1) isolates device time from
    the ~100ms axon dispatch/transfer overhead.

    mode: timing-only ablations -- "full", "nostt" (skip DVE reduce),
    "noexp" (matmul+DMA only), "dmaonly", "nodma" (compute only)."""
    nc = bacc.Bacc("TRN2", target_bir_lowering=False, debug=False,
                   num_devices=NCORES)

    padw = _LAYOUT["padw"]
    str0 = _LAYOUT["str0"]
    s120 = _LAYOUT.get("s120", False)
    if s120:
        blob_d = nc.dram_tensor("blob", (128, str0), BF16,
                                kind="ExternalInput").ap()
        strm_d = nc.dram_tensor("strm", (CROWS, NPAIR * padw), BF16,
                                kind="ExternalInput").ap()
    else:
        blob_d = nc.dram_tensor("blob", (128, _LAYOUT["totw"]), BF16,
                                kind="ExternalInput").ap()
    out_d = nc.dram_tensor("out_s", (128, 2 * NPAIR), F32,
                           kind="ExternalOutput").ap()

    if mode.startswith(("actb", "dveb", "mmb", "ttrb")):
        return _build_microbench(nc, blob_d, out_d, mode, loop_reps)

    # mode string: base[+flags]; flags: sb (single buffer), p4 (psum bufs=4),
    # e6 (epool bufs=6), fine (finer DMA supers), na (drop exp accum; timing)
    parts = mode.split("+")
    mode = parts[0]
    flags = set(parts[1:])
    nbuf = 1 if (loop_reps == 1 or "sb" in flags) else 2
    ps_bufs = 4 if "p4" in flags else 3
    ep_bufs = 8 if "e8" in flags else (6 if "e6" in flags else 4)
    sup_pairs = SUPER_PAIRS
    if "fine" in flags:
        sup_pairs = (1, 2, 3, 4, 5, 6, 7, 4)
    if "crs" in flags:
        sup_pairs = (4, 6, 8, 7, 7)
    use_accum = "na" not in flags
    split_consts = "spl" in flags
    skip_out = "noo" in flags
    res_split = "rsp" in flags
    npair = nblk // 2
    # stream super ranges [p0, p1)
    supers = []
    p0 = 0
    for np_ in sup_pairs:
        if p0 >= npair:
            break
        np_ = min(np_, npair - p0)
        supers.append((p0, p0 + np_))
        p0 += np_

    with tile.TileContext(nc) as tc:
        with (
            tc.tile_pool(name="consts", bufs=1) as consts,
            tc.tile_pool(name="epool", bufs=ep_bufs) as epool,
            tc.tile_pool(name="ps", bufs=ps_bufs, space="PSUM") as ps,
            tc.tile_pool(name="accp", bufs=1, space="PSUM") as accp,
        ):
            g_bufs = [consts.tile([128, _LAYOUT["totw"]], BF16,
                                  name=f"g{i}") for i in range(nbuf)]
            res_bufs = [consts.tile([128, NPAIR], F32, name=f"r{i}")
                        for i in range(nbuf)]
            den_ps_bufs = [accp.tile([128, NPAIR], F32, name=f"dp{i}")
                           for i in range(nbuf)]
            den_sb_bufs = [consts.tile([128, NPAIR], F32, name=f"ds{i}")
                           for i in range(nbuf)]
            scr = consts.tile([128, PW], BF16)
            warm = consts.tile([128, 1], F32)

            # preload the exp table set before the main loop
            nc.vector.memset(warm, 0.0)
            nc.scalar.activation(warm, warm,
                                 mybir.ActivationFunctionType.Exp)
            if mode in ("dmaonly", "noexp", "nostt"):
                for r in res_bufs:
                    nc.vector.memset(r, 1.0)
                for r in den_ps_bufs:
                    nc.vector.memset(r, 1.0)
            if mode == "nodma":
                for g in g_bufs:
                    nc.vector.memset(g, 0.0)

            def main_pass(g_all, res, den_ps, den_sb):
                lhsT = g_all[0:CROWS, LHS0:LHS0 + 128]
                if mode != "nodma":
                    if split_consts:
                        nc.sync.dma_start(g_all[:, 0:128], blob_d[:, 0:128])
                        nc.sync.dma_start(g_all[:, 128:str0],
                                          blob_d[:, 128:str0])
                    else:
                        nc.sync.dma_start(g_all[:, 0:str0],
                                          blob_d[:, 0:str0])
                    for (q0, q1) in supers:
                        if s120:
                            nc.sync.dma_start(
                                g_all[0:CROWS,
                                      str0 + q0 * padw:str0 + q1 * padw],
                                strm_d[:, q0 * padw:q1 * padw])
                        else:
                            c0 = str0 + q0 * padw
                            c1 = str0 + q1 * padw
                            nc.sync.dma_start(g_all[:, c0:c1],
                                              blob_d[:, c0:c1])
                for pp in range(npair):
                    if mode == "dmaonly":
                        break
                    b = pp // (npair // B)
                    col0 = str0 + pp * padw
                    # PSUM tile kept at full 1024 cols so each tile (and the
                    # 512-col matmul split) stays PSUM-bank aligned
                    l_ps = ps.tile([128, PW], F32, tag="l")
                    for hh in range(0, padw, 512):
                        he = min(hh + 512, padw)
                        nc.tensor.matmul(
                            l_ps[:, hh:he], lhsT=lhsT,
                            rhs=g_all[0:CROWS, col0 + hh:col0 + he],
                            start=True, stop=True)
                    if mode == "noexp":
                        continue
                    e_t = epool.tile([128, padw], BF16, tag="e")
                    nc.scalar.activation(e_t, l_ps[:, 0:padw],
                                         mybir.ActivationFunctionType.Exp,
                                         accum_out=(den_ps[:, pp:pp + 1]
                                                    if use_accum else None))
                    if mode == "nostt":
                        continue
                    yt_b = g_all[:, YT0 + b * padw:YT0 + (b + 1) * padw]
                    nc.vector.scalar_tensor_tensor(
                        scr[:, 0:padw], e_t, 0.0, yt_b,
                        op0=mybir.AluOpType.bypass,
                        op1=mybir.AluOpType.mult,
                        accum_out=res[:, pp:pp + 1])
                if not skip_out:
                    nc.vector.tensor_copy(den_sb, den_ps)
                    nc.sync.dma_start(out_d[:, 0:NPAIR], den_sb)
                    nc.sync.dma_start(out_d[:, NPAIR:2 * NPAIR], res)

            if loop_reps > 1:
                if nbuf == 2:
                    assert loop_reps % 2 == 0, "loop_reps must be even"
                with tc.For_i(0, loop_reps // nbuf, 1,
                              hint_engines=(mybir.EngineType.PE,
                                            mybir.EngineType.Activation,
                                            mybir.EngineType.DVE,
                                            mybir.EngineType.SP)):
                    for ib in range(nbuf):
                        main_pass(g_bufs[ib], res_bufs[ib],
                                  den_ps_bufs[ib], den_sb_bufs[ib])
            else:
                main_pass(g_bufs[0], res_bufs[0], den_ps_bufs[0],
                          den_sb_bufs[0])

    nc.compile()
    return nc


def _build_microbench(nc, blob_d, out_d, mode, loop_reps):
    """Pure per-engine instruction pacing benches: NI dependency-free
    instructions per iteration on one engine (same-engine WAW only)."""
    NI = 128
    width = 2048 if "2048" in mode else (1024 if "1024" in mode else 512)
    accum = "na" not in mode
    edt = mybir.dt.float8e4 if "f8" in mode else BF16
    accum_psum = mode.endswith("p")
    with tile.TileContext(nc) as tc:
        with (
            tc.tile_pool(name="consts", bufs=1) as consts,
            tc.tile_pool(name="epool", bufs=4) as epool,
            tc.tile_pool(name="ps", bufs=4, space="PSUM") as ps,
            tc.tile_pool(name="psc", bufs=1, space="PSUM") as psc,
        ):
            g_all = consts.tile([128, 4096], BF16)
            den_buf = consts.tile([128, NBLK], F32)
            num_buf = consts.tile([128, NBLK], F32)
            scr = consts.tile([128, width], edt)
            e_src = consts.tile([128, width], edt)
            warm = consts.tile([128, 1], F32)
            nc.vector.memset(warm, 0.0)
            nc.scalar.activation(warm, warm,
                                 mybir.ActivationFunctionType.Exp)
            nc.vector.memset(g_all, 0.01)
            nc.vector.memset(e_src, 1.0)
            nc.vector.memset(den_buf, 1.0)
            nc.vector.memset(num_buf, 1.0)
            if accum_psum:
                den_ps = psc.tile([128, NBLK], F32)
            l_ps = psc.tile([128, width], F32)
            for h in range(0, width, 512):
                nc.tensor.matmul(l_ps[:, h:h + 512], lhsT=g_all[:, 0:128],
                                 rhs=g_all[:, 128 + h:640 + h],
                                 start=True, stop=True)

            def body():
                for k in range(NI):
                    if mode.startswith("actb"):
                        e_t = epool.tile([128, width], edt, tag="e")
                        tgt = den_ps if accum_psum else den_buf
                        nc.scalar.activation(
                            e_t, l_ps, mybir.ActivationFunctionType.Exp,
                            accum_out=(tgt[:, k % NBLK:k % NBLK + 1]
                                       if accum else None))
                    elif mode.startswith("dveb"):
                        if "m" in mode.split("b")[1]:
                            nc.vector.scalar_tensor_tensor(
                                scr, e_src, 1.0, g_all[:, 128:128 + width],
                                op0=mybir.AluOpType.mult,
                                op1=mybir.AluOpType.mult,
                                accum_out=num_buf[:, k % NBLK:k % NBLK + 1])
                        elif "t" in mode.split("b")[1]:
                            nc.vector.tensor_tensor(
                                scr, e_src, g_all[:, 128:128 + width],
                                op=mybir.AluOpType.mult)
                        else:
                            nc.vector.scalar_tensor_tensor(
                                scr, e_src, 0.0, g_all[:, 128:128 + width],
                                op0=mybir.AluOpType.bypass,
                                op1=mybir.AluOpType.mult,
                                accum_out=num_buf[:, k % NBLK:k % NBLK + 1])
                    elif mode.startswith("ttrb"):
                        nc.vector.tensor_tensor_reduce(
                            scr, e_src, g_all[:, 128:128 + width],
                            1.0, 0.0,
                            op0=mybir.AluOpType.mult,
                            op1=mybir.AluOpType.add,
                            accum_out=num_buf[:, k % NBLK:k % NBLK + 1])
                    else:  # mmb
                        o = ps.tile([128, 512], F32, tag="l")
                        nc.tensor.matmul(o, lhsT=g_all[:, 0:128],
                                         rhs=g_all[:, 128:640],
                                         start=True, stop=True)
                nc.sync.dma_start(out_d[:, 0:NPAIR], den_buf[:, 0:NPAIR])
                nc.sync.dma_start(out_d[:, NPAIR:2 * NPAIR],
                                  num_buf[:, 0:NPAIR])

            if loop_reps > 1:
                with tc.For_i(0, loop_reps, 1,
                              hint_engines=(mybir.EngineType.PE,
                                            mybir.EngineType.Activation,
                                            mybir.EngineType.DVE,
                                            mybir.EngineType.SP)):
                    body()
            else:
                body()
    nc.compile()
    return nc


def _get_program(nblk=NBLK, loop_reps=1, mode="full"):
    key = ("nc", nblk, loop_reps, mode, _LAYOUT["padw"],
           _LAYOUT.get("s120", False))
    if key not in _PROGRAM_CACHE:
        _PROGRAM_CACHE[key] = _build_program(nblk, loop_reps, mode)
    return _PROGRAM_CACHE[key]


def _host_prep(pairwise_g, coset_functions, mask, w_y, w_g):
    """Build the per-core bf16 input blobs with a mask-compacted key axis.

    The free (key) axis is the flat (s2, j) list of mask-valid columns per
    batch, padded to a static width padw; pad columns carry L = -1e30 so
    their exp is exactly 0.  Order is irrelevant (den/num are plain sums).
    """
    y = np.asarray(coset_functions, dtype=np.float32)    # (B, N, S, C)
    maskb = np.asarray(mask)
    w_y0 = np.asarray(w_y, dtype=np.float32)[:, 0]
    w_g = np.asarray(w_g, dtype=np.float32)

    # valid flat key indices v = s2*N + j, per batch
    mflat = maskb.transpose(0, 2, 1).reshape(B, S * N)
    counts = mflat.sum(axis=1)
    padw = max(32, int(-(-int(counts.max()) // 32) * 32))
    str0 = YT0 + B * padw
    totw = str0 + NPAIR * padw
    _LAYOUT.update(padw=padw, str0=str0, totw=totw)

    gidx = np.zeros((B, padw), np.int64)
    pad = np.ones((B, padw), bool)
    for b in range(B):
        ix = np.flatnonzero(mflat[b])
        gidx[b, :len(ix)] = ix
        pad[b, :len(ix)] = False

    # ycols[b, v, c] = y[b, j(v), s2(v), c]
    yv = y.transpose(0, 2, 1, 3).reshape(B, S * N, CIN)
    ycols = np.stack([yv[b, gidx[b]] for b in range(B)])  # (B, padw, C)
    # L rows (C, padw): w_y0*y on valid cols, -1e30 on pads
    ld = w_y0 * ycols
    ld[pad] = -1e30
    ld = np.ascontiguousarray(ld.transpose(0, 2, 1))      # (B, C, padw)
    # y table, zeroed on pads, duplicated over (i2, s1)
    ytab = np.where(pad[..., None], 0.0, ycols).transpose(0, 2, 1)
    yt = np.broadcast_to(ytab[:, None], (B, 16, CIN, padw))
    yt_plane = yt.reshape(B, 128, padw).transpose(1, 0, 2).reshape(128, -1)

    # stationary lhsT (128, 128): out col k = i2*64 + s1*8 + c
    lhsT = np.zeros((128, 128), np.float32)
    for i2 in range(2):
        for s1 in range(S):
            for g in range(GDIM):
                p = i2 * 56 + s1 * GDIM + g
                k0 = i2 * 64 + s1 * 8
                lhsT[p, k0:k0 + CIN] = w_g[:, g]
    for c in range(CIN):
        for i2 in range(2):
            for s1 in range(S):
                lhsT[112 + c, i2 * 64 + s1 * 8 + c] = 1.0

    consts_plane = np.empty((128, str0), NPBF16)
    consts_plane[:, LHS0:LHS0 + 128] = lhsT
    consts_plane[:, YT0:YT0 + B * padw] = yt_plane

    pairwise_g = np.asarray(pairwise_g, dtype=np.float32)
    in_maps = []
    for k in range(NCORES):
        sl = slice(ISHARD * k, ISHARD * (k + 1))
        pg = pairwise_g[:, sl]                   # (B, 16, j, s1, s2, g)
        x = pg.reshape(B, 8, 2, N, S, S, GDIM)
        x = x.transpose(0, 1, 2, 4, 6, 5, 3)     # [b,ip,i2,s1,g,s2,j]
        x = np.ascontiguousarray(x).reshape(B, 8, 2, S, GDIM, S * N)
        xg = np.stack([x[b][..., gidx[b]] for b in range(B)])
        xg = xg.reshape(NPAIR, 112, padw)
        if _LAYOUT.get("s120", False):
            strm = np.empty((CROWS, NPAIR, padw), NPBF16)
            strm[0:112] = xg.transpose(1, 0, 2)
            strm[112:CROWS] = np.repeat(ld, NPAIR // B, axis=0) \
                .transpose(1, 0, 2)
            in_maps.append({"blob": np.ascontiguousarray(consts_plane),
                            "strm": strm.reshape(CROWS, -1)})
        else:
            blob = np.empty((128, totw), NPBF16)
            blob[:, :str0] = consts_plane
            stream = blob[:, str0:].reshape(128, NPAIR, padw)
            stream[0:112] = xg.transpose(1, 0, 2)
            stream[112:CROWS] = np.repeat(ld, NPAIR // B, axis=0) \
                .transpose(1, 0, 2)
            in_maps.append({"blob": blob})
    return in_maps


def _host_finish(s_list, coset_functions, mask, w_lin):
    """Decode per-core (128, 64) outputs into the full result."""
    y = np.asarray(coset_functions, dtype=np.float32)
    maskf = np.asarray(mask).astype(np.float32)
    w_lin = np.asarray(w_lin, dtype=np.float32)
    out = np.empty((B, N, S, COUT), np.float32)
    for k in range(NCORES):
        s = np.asarray(s_list[k], dtype=np.float32)      # (128, 2*NPAIR)
        # partition p = i2*64 + s1*8 + c; block = 2*pp + i2
        den = s[:, :NPAIR].reshape(2, S, CIN, NPAIR).transpose(3, 0, 1, 2)
        num = s[:, NPAIR:].reshape(2, S, CIN, NPAIR).transpose(3, 0, 1, 2)
        den = den.reshape(NBLK, S, CIN)
        num = num.reshape(NBLK, S, CIN)
        sl = slice(ISHARD * k, ISHARD * (k + 1))
        y_q = y[:, sl].reshape(NBLK, S, CIN)
        m_q = maskf[:, sl].reshape(NBLK, S)
        res = (y_q + num / den) * m_q[..., None]
        res = res @ w_lin.T
        out[:, sl] = res.reshape(B, ISHARD, S, COUT)
    return out


def kernel(pairwise_g, coset_functions, mask, w_y, b_y, w_g, b_g, w_lin):
    pairwise_g = np.asarray(pairwise_g)
    coset_functions = np.asarray(coset_functions)
    mask = np.asarray(mask)

    in_maps = _host_prep(pairwise_g, coset_functions, mask,
                         np.asarray(w_y), np.asarray(w_g))
    nc = _get_program()
    res = run_bass_kernel_spmd(nc, in_maps, core_ids=list(range(NCORES)))
    s_list = [r["out_s"] for r in res.results]
    return _host_finish(s_list, coset_functions, mask, np.asarray(w_lin))


# revision 32
# speedup vs baseline: 1.0216x; 1.0216x over previous
"""Trainium2 Bass kernel for equivariant multihead attention.

Math (per batch b, query point i, coset s1, channel c):
    logit[j,s2] = sum_g pairwise_g[b,i,j,s1,s2,g]*w_g[c,g]
                  + w_y[c,0]*y[b,j,s2,c] + w_y[c,1]*y[b,i,s1,c] + b_g[c] + b_y[c]
    att = exp(logit)*mask[b,j,s2];  att /= sum_{j,s2} att
    out = (y[b,i,s1,c] + sum_{j,s2} att*y[b,j,s2,c]) * mask[b,i,s1]  @ w_lin.T

The query-side term and the biases are constant over the key dims (j,s2) and
cancel in the normalization, so they are dropped.  The key-side factor
exp(w_y[c,0]*y)*mask is folded INTO the exponent: with
    L[(s2,c), j] = w_y0[c]*y[b,j,s2,c] + log(mask[b,j,s2])   (-1e30 if masked)
the unnormalized attention is E' = exp(sum_g pg*w_g + L) directly, and
    den[s1,c] = sum_{j,s2} E'               (free-dim accumulate during exp)
    num[s1,c] = sum_{j,s2} E' * y[b,j,s2,c] (one fused multiply-reduce)

TWO query points (a "pair") are packed per instruction: output partition is
(i2, s1, c) = 2*8*8 = 128 and the free dim is (h, t, j) = 2*4*128 = 1024
(s2 = 2t+h lives entirely in the free dim, so the per-partition accumulate
gives complete per-(i,s1,c) sums).  Device work per pair is FOUR instructions:
  * two 120x128 @ 120x512 matmuls (contraction: 112 pairwise-g rows for both
    i2 + 8 L-rows added via an indicator block in the fixed stationary),
  * exp on the Act engine (128x1024) with accum_out -> den column,
  * one scalar_tensor_tensor against a y-table with accum_out -> num column.
The host pre-transposes pairwise_g into the rhs stream (bf16, halves DMA) and
finishes with the residual add, query mask, and the c_in->c_out linear.

Sharding: query dim i is split 8 ways (16 i x 4 b = 32 pairs per core).
"""

import numpy as np

import concourse.bacc as bacc
import concourse.tile as tile
from concourse import mybir
from concourse.bass_utils import run_bass_kernel_spmd

B, N, S, CIN, COUT, GDIM = 4, 128, 8, 8, 8, 7
NCORES = 8
ISHARD = N // NCORES          # 16 query points per core
NBLK = B * ISHARD             # 64 (b,i) blocks per core
NPAIR = NBLK // 2             # 32 block pairs per core
NT = 4                        # s2-pair tiles
PW = 2 * NT * N               # 1024 rhs columns per pair: (h, t, j)
CROWS = 120                   # contraction rows: 112 pairwise-g + 8 L

# blob column layout (bf16): [lhsT | YT tables | rhs stream].  The key
# axis (s2, j) is COMPACTED per batch to the mask-valid columns (masked
# keys contribute exactly 0), padded to a static width padw; _host_prep
# computes the layout from the actual mask and updates _LAYOUT.
LHS0 = 0
YT0 = 128
STR0 = YT0 + B * PW           # 4224 (dense fallback)
TOTW = STR0 + NPAIR * PW      # 36992 (dense fallback)
_LAYOUT = {"padw": PW, "str0": STR0, "totw": TOTW}

# pairs covered by each stream DMA (earlier ones smaller for pipeline ramp)
SUPER_PAIRS = (2, 4, 5, 6, 7, 8)

F32 = mybir.dt.float32
BF16 = mybir.dt.bfloat16
NPBF16 = mybir.dt.np(mybir.dt.bfloat16)

_PROGRAM_CACHE = {}


def _build_program(nblk=NBLK, loop_reps=1, mode="full"):
    """loop_reps>1 wraps the main loop in a hardware For_i that re-runs the
    full pass (including the input DMAs) on the same data -- used only for
    timing: wall(loop_reps=R) - wall(loop_reps=1) isolates device time from
    the ~100ms axon dispatch/transfer overhead.

    mode: timing-only ablations -- "full", "nostt" (skip DVE reduce),
    "noexp" (matmul+DMA only), "dmaonly", "nodma" (compute only)."""
    nc = bacc.Bacc("TRN2", target_bir_lowering=False, debug=False,
                   num_devices=NCORES)

    padw = _LAYOUT["padw"]
    str0 = _LAYOUT["str0"]
    s120 = _LAYOUT.get("s120", False)
    if s120:
        blob_d = nc.dram_tensor("blob", (128, str0), BF16,
                                kind="ExternalInput").ap()
        strm_d = nc.dram_tensor("strm", (CROWS, NPAIR * padw), BF16,
                                kind="ExternalInput").ap()
    else:
        blob_d = nc.dram_tensor("blob", (128, _LAYOUT["totw"]), BF16,
                                kind="ExternalInput").ap()
    out_d = nc.dram_tensor("out_s", (128, 2 * NPAIR), F32,
                           kind="ExternalOutput").ap()

    if mode.startswith(("actb", "dveb", "mmb", "ttrb")):
        return _build_microbench(nc, blob_d, out_d, mode, loop_reps)

    # mode string: base[+flags]; flags: sb (single buffer), p4 (psum bufs=4),
    # e6 (epool bufs=6), fine (finer DMA supers), na (drop exp accum; timing)
    parts = mode.split("+")
    mode = parts[0]
    flags = set(parts[1:])
    nbuf = 1 if (loop_reps == 1 or "sb" in flags) else 2
    ps_bufs = 4 if "p4" in flags else 3
    ep_bufs = 8 if "e8" in flags else (6 if "e6" in flags else 4)
    sup_pairs = SUPER_PAIRS
    if "fine" in flags:
        sup_pairs = (1, 2, 3, 4, 5, 6, 7, 4)
    if "crs" in flags:
        sup_pairs = (4, 6, 8, 7, 7)
    use_accum = "na" not in flags
    split_consts = "spl" in flags
    skip_out = "noo" in flags
    res_split = "rsp" in flags
    npair = nblk // 2
    # stream super ranges [p0, p1)
    supers = []
    p0 = 0
    for np_ in sup_pairs:
        if p0 >= npair:
            break
        np_ = min(np_, npair - p0)
        supers.append((p0, p0 + np_))
        p0 += np_

    with tile.TileContext(nc) as tc:
        with (
            tc.tile_pool(name="consts", bufs=1) as consts,
            tc.tile_pool(name="epool", bufs=ep_bufs) as epool,
            tc.tile_pool(name="ps", bufs=ps_bufs, space="PSUM") as ps,
            tc.tile_pool(name="accp", bufs=1, space="PSUM") as accp,
        ):
            g_bufs = [consts.tile([128, _LAYOUT["totw"]], BF16,
                                  name=f"g{i}") for i in range(nbuf)]
            res_bufs = [consts.tile([128, NPAIR], F32, name=f"r{i}")
                        for i in range(nbuf)]
            den_ps_bufs = [accp.tile([128, NPAIR], F32, name=f"dp{i}")
                           for i in range(nbuf)]
            den_sb_bufs = [consts.tile([128, NPAIR], F32, name=f"ds{i}")
                           for i in range(nbuf)]
            scr = consts.tile([128, PW], BF16)
            warm = consts.tile([128, 1], F32)

            # preload the exp table set before the main loop
            nc.vector.memset(warm, 0.0)
            nc.scalar.activation(warm, warm,
                                 mybir.ActivationFunctionType.Exp)
            if mode in ("dmaonly", "noexp", "nostt"):
                for r in res_bufs:
                    nc.vector.memset(r, 1.0)
                for r in den_ps_bufs:
                    nc.vector.memset(r, 1.0)
            if mode == "nodma":
                for g in g_bufs:
                    nc.vector.memset(g, 0.0)

            def main_pass(g_all, res, den_ps, den_sb):
                lhsT = g_all[0:CROWS, LHS0:LHS0 + 128]
                if mode != "nodma":
                    if split_consts:
                        nc.sync.dma_start(g_all[:, 0:128], blob_d[:, 0:128])
                        nc.sync.dma_start(g_all[:, 128:str0],
                                          blob_d[:, 128:str0])
                    else:
                        nc.sync.dma_start(g_all[:, 0:str0],
                                          blob_d[:, 0:str0])
                    for (q0, q1) in supers:
                        if s120:
                            nc.sync.dma_start(
                                g_all[0:CROWS,
                                      str0 + q0 * padw:str0 + q1 * padw],
                                strm_d[:, q0 * padw:q1 * padw])
                        else:
                            c0 = str0 + q0 * padw
                            c1 = str0 + q1 * padw
                            nc.sync.dma_start(g_all[:, c0:c1],
                                              blob_d[:, c0:c1])
                for pp in range(npair):
                    if mode == "dmaonly":
                        break
                    b = pp // (npair // B)
                    col0 = str0 + pp * padw
                    # PSUM tile kept at full 1024 cols so each tile (and the
                    # 512-col matmul split) stays PSUM-bank aligned
                    l_ps = ps.tile([128, PW], F32, tag="l")
                    for hh in range(0, padw, 512):
                        he = min(hh + 512, padw)
                        nc.tensor.matmul(
                            l_ps[:, hh:he], lhsT=lhsT,
                            rhs=g_all[0:CROWS, col0 + hh:col0 + he],
                            start=True, stop=True)
                    if mode == "noexp":
                        continue
                    e_t = epool.tile([128, padw], BF16, tag="e")
                    nc.scalar.activation(e_t, l_ps[:, 0:padw],
                                         mybir.ActivationFunctionType.Exp,
                                         accum_out=(den_ps[:, pp:pp + 1]
                                                    if use_accum else None))
                    if mode == "nostt":
                        continue
                    yt_b = g_all[:, YT0 + b * padw:YT0 + (b + 1) * padw]
                    nc.vector.scalar_tensor_tensor(
                        scr[:, 0:padw], e_t, 0.0, yt_b,
                        op0=mybir.AluOpType.bypass,
                        op1=mybir.AluOpType.mult,
                        accum_out=res[:, pp:pp + 1])
                if not skip_out:
                    nc.vector.tensor_copy(den_sb, den_ps)
                    nc.sync.dma_start(out_d[:, 0:NPAIR], den_sb)
                    nc.sync.dma_start(out_d[:, NPAIR:2 * NPAIR], res)

            if loop_reps > 1:
                if nbuf == 2:
                    assert loop_reps % 2 == 0, "loop_reps must be even"
                with tc.For_i(0, loop_reps // nbuf, 1,
                              hint_engines=(mybir.EngineType.PE,
                                            mybir.EngineType.Activation,
                                            mybir.EngineType.DVE,
                                            mybir.EngineType.SP)):
                    for ib in range(nbuf):
                        main_pass(g_bufs[ib], res_bufs[ib],
                                  den_ps_bufs[ib], den_sb_bufs[ib])
            else:
                main_pass(g_bufs[0], res_bufs[0], den_ps_bufs[0],
                          den_sb_bufs[0])

    nc.compile()
    return nc


def _build_microbench(nc, blob_d, out_d, mode, loop_reps):
    """Pure per-engine instruction pacing benches: NI dependency-free
    instructions per iteration on one engine (same-engine WAW only)."""
    NI = 128
    width = 2048 if "2048" in mode else (1024 if "1024" in mode else 512)
    accum = "na" not in mode
    edt = mybir.dt.float8e4 if "f8" in mode else BF16
    accum_psum = mode.endswith("p")
    with tile.TileContext(nc) as tc:
        with (
            tc.tile_pool(name="consts", bufs=1) as consts,
            tc.tile_pool(name="epool", bufs=4) as epool,
            tc.tile_pool(name="ps", bufs=4, space="PSUM") as ps,
            tc.tile_pool(name="psc", bufs=1, space="PSUM") as psc,
        ):
            g_all = consts.tile([128, 4096], BF16)
            den_buf = consts.tile([128, NBLK], F32)
            num_buf = consts.tile([128, NBLK], F32)
            scr = consts.tile([128, width], edt)
            e_src = consts.tile([128, width], edt)
            warm = consts.tile([128, 1], F32)
            nc.vector.memset(warm, 0.0)
            nc.scalar.activation(warm, warm,
                                 mybir.ActivationFunctionType.Exp)
            nc.vector.memset(g_all, 0.01)
            nc.vector.memset(e_src, 1.0)
            nc.vector.memset(den_buf, 1.0)
            nc.vector.memset(num_buf, 1.0)
            if accum_psum:
                den_ps = psc.tile([128, NBLK], F32)
            l_ps = psc.tile([128, width], F32)
            for h in range(0, width, 512):
                nc.tensor.matmul(l_ps[:, h:h + 512], lhsT=g_all[:, 0:128],
                                 rhs=g_all[:, 128 + h:640 + h],
                                 start=True, stop=True)

            def body():
                for k in range(NI):
                    if mode.startswith("actb"):
                        e_t = epool.tile([128, width], edt, tag="e")
                        tgt = den_ps if accum_psum else den_buf
                        nc.scalar.activation(
                            e_t, l_ps, mybir.ActivationFunctionType.Exp,
                            accum_out=(tgt[:, k % NBLK:k % NBLK + 1]
                                       if accum else None))
                    elif mode.startswith("dveb"):
                        if accum_psum:
                            nc.vector.scalar_tensor_tensor(
                                scr, e_src, 0.0, g_all[:, 128:128 + width],
                                op0=mybir.AluOpType.bypass,
                                op1=mybir.AluOpType.mult,
                                accum_out=den_ps[:, k % NBLK:k % NBLK + 1])
                        elif "m" in mode.split("b")[1]:
                            nc.vector.scalar_tensor_tensor(
                                scr, e_src, 1.0, g_all[:, 128:128 + width],
                                op0=mybir.AluOpType.mult,
                                op1=mybir.AluOpType.mult,
                                accum_out=num_buf[:, k % NBLK:k % NBLK + 1])
                        elif "t" in mode.split("b")[1]:
                            nc.vector.tensor_tensor(
                                scr, e_src, g_all[:, 128:128 + width],
                                op=mybir.AluOpType.mult)
                        else:
                            nc.vector.scalar_tensor_tensor(
                                scr, e_src, 0.0, g_all[:, 128:128 + width],
                                op0=mybir.AluOpType.bypass,
                                op1=mybir.AluOpType.mult,
                                accum_out=num_buf[:, k % NBLK:k % NBLK + 1])
                    elif mode.startswith("ttrb"):
                        nc.vector.tensor_tensor_reduce(
                            scr, e_src, g_all[:, 128:128 + width],
                            1.0, 0.0,
                            op0=mybir.AluOpType.mult,
                            op1=mybir.AluOpType.add,
                            accum_out=num_buf[:, k % NBLK:k % NBLK + 1])
                    else:  # mmb
                        o = ps.tile([128, 512], F32, tag="l")
                        nc.tensor.matmul(o, lhsT=g_all[:, 0:128],
                                         rhs=g_all[:, 128:640],
                                         start=True, stop=True)
                nc.sync.dma_start(out_d[:, 0:NPAIR], den_buf[:, 0:NPAIR])
                nc.sync.dma_start(out_d[:, NPAIR:2 * NPAIR],
                                  num_buf[:, 0:NPAIR])

            if loop_reps > 1:
                with tc.For_i(0, loop_reps, 1,
                              hint_engines=(mybir.EngineType.PE,
                                            mybir.EngineType.Activation,
                                            mybir.EngineType.DVE,
                                            mybir.EngineType.SP)):
                    body()
            else:
                body()
    nc.compile()
    return nc


def _get_program(nblk=NBLK, loop_reps=1, mode="full"):
    key = ("nc", nblk, loop_reps, mode, _LAYOUT["padw"],
           _LAYOUT.get("s120", False))
    if key not in _PROGRAM_CACHE:
        _PROGRAM_CACHE[key] = _build_program(nblk, loop_reps, mode)
    return _PROGRAM_CACHE[key]


def _host_prep(pairwise_g, coset_functions, mask, w_y, w_g):
    """Build the per-core bf16 input blobs with a mask-compacted key axis.

    The free (key) axis is the flat (s2, j) list of mask-valid columns per
    batch, padded to a static width padw; pad columns carry L = -1e30 so
    their exp is exactly 0.  Order is irrelevant (den/num are plain sums).
    """
    y = np.asarray(coset_functions, dtype=np.float32)    # (B, N, S, C)
    maskb = np.asarray(mask)
    w_y0 = np.asarray(w_y, dtype=np.float32)[:, 0]
    w_g = np.asarray(w_g, dtype=np.float32)

    # valid flat key indices v = s2*N + j, per batch
    mflat = maskb.transpose(0, 2, 1).reshape(B, S * N)
    counts = mflat.sum(axis=1)
    padw = max(32, int(-(-int(counts.max()) // 32) * 32))
    str0 = YT0 + B * padw
    totw = str0 + NPAIR * padw
    _LAYOUT.update(padw=padw, str0=str0, totw=totw)

    gidx = np.zeros((B, padw), np.int64)
    pad = np.ones((B, padw), bool)
    for b in range(B):
        ix = np.flatnonzero(mflat[b])
        gidx[b, :len(ix)] = ix
        pad[b, :len(ix)] = False

    # ycols[b, v, c] = y[b, j(v), s2(v), c]
    yv = y.transpose(0, 2, 1, 3).reshape(B, S * N, CIN)
    ycols = np.stack([yv[b, gidx[b]] for b in range(B)])  # (B, padw, C)
    # L rows (C, padw): w_y0*y on valid cols, -1e30 on pads
    ld = w_y0 * ycols
    ld[pad] = -1e30
    ld = np.ascontiguousarray(ld.transpose(0, 2, 1))      # (B, C, padw)
    # y table, zeroed on pads, duplicated over (i2, s1)
    ytab = np.where(pad[..., None], 0.0, ycols).transpose(0, 2, 1)
    yt = np.broadcast_to(ytab[:, None], (B, 16, CIN, padw))
    yt_plane = yt.reshape(B, 128, padw).transpose(1, 0, 2).reshape(128, -1)

    # stationary lhsT (128, 128): out col k = i2*64 + s1*8 + c
    lhsT = np.zeros((128, 128), np.float32)
    for i2 in range(2):
        for s1 in range(S):
            for g in range(GDIM):
                p = i2 * 56 + s1 * GDIM + g
                k0 = i2 * 64 + s1 * 8
                lhsT[p, k0:k0 + CIN] = w_g[:, g]
    for c in range(CIN):
        for i2 in range(2):
            for s1 in range(S):
                lhsT[112 + c, i2 * 64 + s1 * 8 + c] = 1.0

    consts_plane = np.empty((128, str0), NPBF16)
    consts_plane[:, LHS0:LHS0 + 128] = lhsT
    consts_plane[:, YT0:YT0 + B * padw] = yt_plane

    pairwise_g = np.asarray(pairwise_g, dtype=np.float32)
    in_maps = []
    for k in range(NCORES):
        sl = slice(ISHARD * k, ISHARD * (k + 1))
        pg = pairwise_g[:, sl]                   # (B, 16, j, s1, s2, g)
        x = pg.reshape(B, 8, 2, N, S, S, GDIM)
        x = x.transpose(0, 1, 2, 4, 6, 5, 3)     # [b,ip,i2,s1,g,s2,j]
        x = np.ascontiguousarray(x).reshape(B, 8, 2, S, GDIM, S * N)
        xg = np.stack([x[b][..., gidx[b]] for b in range(B)])
        xg = xg.reshape(NPAIR, 112, padw)
        if _LAYOUT.get("s120", False):
            strm = np.empty((CROWS, NPAIR, padw), NPBF16)
            strm[0:112] = xg.transpose(1, 0, 2)
            strm[112:CROWS] = np.repeat(ld, NPAIR // B, axis=0) \
                .transpose(1, 0, 2)
            in_maps.append({"blob": np.ascontiguousarray(consts_plane),
                            "strm": strm.reshape(CROWS, -1)})
        else:
            blob = np.empty((128, totw), NPBF16)
            blob[:, :str0] = consts_plane
            stream = blob[:, str0:].reshape(128, NPAIR, padw)
            stream[0:112] = xg.transpose(1, 0, 2)
            stream[112:CROWS] = np.repeat(ld, NPAIR // B, axis=0) \
                .transpose(1, 0, 2)
            in_maps.append({"blob": blob})
    return in_maps


def _host_finish(s_list, coset_functions, mask, w_lin):
    """Decode per-core (128, 64) outputs into the full result."""
    y = np.asarray(coset_functions, dtype=np.float32)
    maskf = np.asarray(mask).astype(np.float32)
    w_lin = np.asarray(w_lin, dtype=np.float32)
    out = np.empty((B, N, S, COUT), np.float32)
    for k in range(NCORES):
        s = np.asarray(s_list[k], dtype=np.float32)      # (128, 2*NPAIR)
        # partition p = i2*64 + s1*8 + c; block = 2*pp + i2
        den = s[:, :NPAIR].reshape(2, S, CIN, NPAIR).transpose(3, 0, 1, 2)
        num = s[:, NPAIR:].reshape(2, S, CIN, NPAIR).transpose(3, 0, 1, 2)
        den = den.reshape(NBLK, S, CIN)
        num = num.reshape(NBLK, S, CIN)
        sl = slice(ISHARD * k, ISHARD * (k + 1))
        y_q = y[:, sl].reshape(NBLK, S, CIN)
        m_q = maskf[:, sl].reshape(NBLK, S)
        res = (y_q + num / den) * m_q[..., None]
        res = res @ w_lin.T
        out[:, sl] = res.reshape(B, ISHARD, S, COUT)
    return out


def kernel(pairwise_g, coset_functions, mask, w_y, b_y, w_g, b_g, w_lin):
    pairwise_g = np.asarray(pairwise_g)
    coset_functions = np.asarray(coset_functions)
    mask = np.asarray(mask)

    in_maps = _host_prep(pairwise_g, coset_functions, mask,
                         np.asarray(w_y), np.asarray(w_g))
    nc = _get_program()
    res = run_bass_kernel_spmd(nc, in_maps, core_ids=list(range(NCORES)))
    s_list = [r["out_s"] for r in res.results]
    return _host_finish(s_list, coset_functions, mask, np.asarray(w_lin))


# revision 33
# speedup vs baseline: 1.0512x; 1.0289x over previous
"""Trainium2 Bass kernel for equivariant multihead attention.

Math (per batch b, query point i, coset s1, channel c):
    logit[j,s2] = sum_g pairwise_g[b,i,j,s1,s2,g]*w_g[c,g]
                  + w_y[c,0]*y[b,j,s2,c] + w_y[c,1]*y[b,i,s1,c] + b_g[c] + b_y[c]
    att = exp(logit)*mask[b,j,s2];  att /= sum_{j,s2} att
    out = (y[b,i,s1,c] + sum_{j,s2} att*y[b,j,s2,c]) * mask[b,i,s1]  @ w_lin.T

The query-side term and the biases are constant over the key dims (j,s2) and
cancel in the normalization, so they are dropped.  The key-side factor
exp(w_y[c,0]*y)*mask is folded INTO the exponent: with
    L[(s2,c), j] = w_y0[c]*y[b,j,s2,c] + log(mask[b,j,s2])   (-1e30 if masked)
the unnormalized attention is E' = exp(sum_g pg*w_g + L) directly, and
    den[s1,c] = sum_{j,s2} E'               (free-dim accumulate during exp)
    num[s1,c] = sum_{j,s2} E' * y[b,j,s2,c] (one fused multiply-reduce)

TWO query points (a "pair") are packed per instruction: output partition is
(i2, s1, c) = 2*8*8 = 128 and the free dim is (h, t, j) = 2*4*128 = 1024
(s2 = 2t+h lives entirely in the free dim, so the per-partition accumulate
gives complete per-(i,s1,c) sums).  Device work per pair is FOUR instructions:
  * two 120x128 @ 120x512 matmuls (contraction: 112 pairwise-g rows for both
    i2 + 8 L-rows added via an indicator block in the fixed stationary),
  * exp on the Act engine (128x1024) with accum_out -> den column,
  * one scalar_tensor_tensor against a y-table with accum_out -> num column.
The host pre-transposes pairwise_g into the rhs stream (bf16, halves DMA) and
finishes with the residual add, query mask, and the c_in->c_out linear.

Sharding: query dim i is split 8 ways (16 i x 4 b = 32 pairs per core).
"""

import numpy as np

import concourse.bacc as bacc
import concourse.tile as tile
from concourse import mybir
from concourse.bass_utils import run_bass_kernel_spmd

B, N, S, CIN, COUT, GDIM = 4, 128, 8, 8, 8, 7
NCORES = 8
ISHARD = N // NCORES          # 16 query points per core
NBLK = B * ISHARD             # 64 (b,i) blocks per core
NPAIR = NBLK // 2             # 32 block pairs per core
NT = 4                        # s2-pair tiles
PW = 2 * NT * N               # 1024 rhs columns per pair: (h, t, j)
CROWS = 120                   # contraction rows: 112 pairwise-g + 8 L

# blob column layout (bf16): [lhsT | YT tables | rhs stream].  The key
# axis (s2, j) is COMPACTED per batch to the mask-valid columns (masked
# keys contribute exactly 0), padded to a static width padw; _host_prep
# computes the layout from the actual mask and updates _LAYOUT.
LHS0 = 0
YT0 = 128
STR0 = YT0 + B * PW           # 4224 (dense fallback)
TOTW = STR0 + NPAIR * PW      # 36992 (dense fallback)
_LAYOUT = {"padw": PW, "str0": STR0, "totw": TOTW}

# pairs covered by each stream DMA (earlier ones smaller for pipeline ramp)
SUPER_PAIRS = (2, 4, 5, 6, 7, 8)

F32 = mybir.dt.float32
BF16 = mybir.dt.bfloat16
NPBF16 = mybir.dt.np(mybir.dt.bfloat16)

_PROGRAM_CACHE = {}


def _build_program(nblk=NBLK, loop_reps=1, mode="full"):
    """loop_reps>1 wraps the main loop in a hardware For_i that re-runs the
    full pass (including the input DMAs) on the same data -- used only for
    timing: wall(loop_reps=R) - wall(loop_reps=1) isolates device time from
    the ~100ms axon dispatch/transfer overhead.

    mode: timing-only ablations -- "full", "nostt" (skip DVE reduce),
    "noexp" (matmul+DMA only), "dmaonly", "nodma" (compute only)."""
    nc = bacc.Bacc("TRN2", target_bir_lowering=False, debug=False,
                   num_devices=NCORES)

    padw = _LAYOUT["padw"]
    str0 = _LAYOUT["str0"]
    s120 = _LAYOUT.get("s120", False)
    if s120:
        blob_d = nc.dram_tensor("blob", (128, str0), BF16,
                                kind="ExternalInput").ap()
        strm_d = nc.dram_tensor("strm", (CROWS, NPAIR * padw), BF16,
                                kind="ExternalInput").ap()
    else:
        blob_d = nc.dram_tensor("blob", (128, _LAYOUT["totw"]), BF16,
                                kind="ExternalInput").ap()
    out_d = nc.dram_tensor("out_s", (128, 2 * NPAIR), F32,
                           kind="ExternalOutput").ap()

    if mode.startswith(("actb", "dveb", "mmb", "ttrb", "redb", "ttb")):
        return _build_microbench(nc, blob_d, out_d, mode, loop_reps)

    # mode string: base[+flags]; flags: sb (single buffer), p4 (psum bufs=4),
    # e6 (epool bufs=6), fine (finer DMA supers), na (drop exp accum; timing)
    parts = mode.split("+")
    mode = parts[0]
    flags = set(parts[1:])
    nbuf = 1 if (loop_reps == 1 or "sb" in flags) else 2
    ps_bufs = 4 if "p4" in flags else 3
    ep_bufs = 8 if "e8" in flags else (6 if "e6" in flags else 4)
    sup_pairs = SUPER_PAIRS
    if "fine" in flags:
        sup_pairs = (1, 2, 3, 4, 5, 6, 7, 4)
    if "crs" in flags:
        sup_pairs = (4, 6, 8, 7, 7)
    use_accum = "na" not in flags
    split_consts = "spl" in flags
    skip_out = "noo" in flags
    res_split = "rsp" in flags
    npair = nblk // 2
    # stream super ranges [p0, p1)
    supers = []
    p0 = 0
    for np_ in sup_pairs:
        if p0 >= npair:
            break
        np_ = min(np_, npair - p0)
        supers.append((p0, p0 + np_))
        p0 += np_

    with tile.TileContext(nc) as tc:
        with (
            tc.tile_pool(name="consts", bufs=1) as consts,
            tc.tile_pool(name="epool", bufs=ep_bufs) as epool,
            tc.tile_pool(name="ps", bufs=ps_bufs, space="PSUM") as ps,
            tc.tile_pool(name="accp", bufs=1, space="PSUM") as accp,
        ):
            g_bufs = [consts.tile([128, _LAYOUT["totw"]], BF16,
                                  name=f"g{i}") for i in range(nbuf)]
            res_bufs = [consts.tile([128, NPAIR], F32, name=f"r{i}")
                        for i in range(nbuf)]
            den_ps_bufs = [accp.tile([128, NPAIR], F32, name=f"dp{i}")
                           for i in range(nbuf)]
            den_sb_bufs = [consts.tile([128, NPAIR], F32, name=f"ds{i}")
                           for i in range(nbuf)]
            scr = consts.tile([128, PW], BF16)
            warm = consts.tile([128, 1], F32)

            # preload the exp table set before the main loop
            nc.vector.memset(warm, 0.0)
            nc.scalar.activation(warm, warm,
                                 mybir.ActivationFunctionType.Exp)
            if mode in ("dmaonly", "noexp", "nostt"):
                for r in res_bufs:
                    nc.vector.memset(r, 1.0)
                for r in den_ps_bufs:
                    nc.vector.memset(r, 1.0)
            if mode == "nodma":
                for g in g_bufs:
                    nc.vector.memset(g, 0.0)

            def main_pass(g_all, res, den_ps, den_sb):
                lhsT = g_all[0:CROWS, LHS0:LHS0 + 128]
                if mode != "nodma":
                    if split_consts:
                        nc.sync.dma_start(g_all[:, 0:128], blob_d[:, 0:128])
                        nc.sync.dma_start(g_all[:, 128:str0],
                                          blob_d[:, 128:str0])
                    else:
                        nc.sync.dma_start(g_all[:, 0:str0],
                                          blob_d[:, 0:str0])
                    for (q0, q1) in supers:
                        if s120:
                            nc.sync.dma_start(
                                g_all[0:CROWS,
                                      str0 + q0 * padw:str0 + q1 * padw],
                                strm_d[:, q0 * padw:q1 * padw])
                        else:
                            c0 = str0 + q0 * padw
                            c1 = str0 + q1 * padw
                            nc.sync.dma_start(g_all[:, c0:c1],
                                              blob_d[:, c0:c1])
                for pp in range(npair):
                    if mode == "dmaonly":
                        break
                    b = pp // (npair // B)
                    col0 = str0 + pp * padw
                    # PSUM tile kept at full 1024 cols so each tile (and the
                    # 512-col matmul split) stays PSUM-bank aligned
                    l_ps = ps.tile([128, PW], F32, tag="l")
                    for hh in range(0, padw, 512):
                        he = min(hh + 512, padw)
                        nc.tensor.matmul(
                            l_ps[:, hh:he], lhsT=lhsT,
                            rhs=g_all[0:CROWS, col0 + hh:col0 + he],
                            start=True, stop=True)
                    if mode == "noexp":
                        continue
                    e_t = epool.tile([128, padw], BF16, tag="e")
                    nc.scalar.activation(e_t, l_ps[:, 0:padw],
                                         mybir.ActivationFunctionType.Exp,
                                         accum_out=(den_ps[:, pp:pp + 1]
                                                    if use_accum else None))
                    if mode == "nostt":
                        continue
                    yt_b = g_all[:, YT0 + b * padw:YT0 + (b + 1) * padw]
                    nc.vector.scalar_tensor_tensor(
                        scr[:, 0:padw], e_t, 0.0, yt_b,
                        op0=mybir.AluOpType.bypass,
                        op1=mybir.AluOpType.mult,
                        accum_out=res[:, pp:pp + 1])
                if not skip_out:
                    nc.vector.tensor_copy(den_sb, den_ps)
                    nc.sync.dma_start(out_d[:, 0:NPAIR], den_sb)
                    nc.sync.dma_start(out_d[:, NPAIR:2 * NPAIR], res)

            if loop_reps > 1:
                if nbuf == 2:
                    assert loop_reps % 2 == 0, "loop_reps must be even"
                with tc.For_i(0, loop_reps // nbuf, 1,
                              hint_engines=(mybir.EngineType.PE,
                                            mybir.EngineType.Activation,
                                            mybir.EngineType.DVE,
                                            mybir.EngineType.SP)):
                    for ib in range(nbuf):
                        main_pass(g_bufs[ib], res_bufs[ib],
                                  den_ps_bufs[ib], den_sb_bufs[ib])
            else:
                main_pass(g_bufs[0], res_bufs[0], den_ps_bufs[0],
                          den_sb_bufs[0])

    nc.compile()
    return nc


def _build_microbench(nc, blob_d, out_d, mode, loop_reps):
    """Pure per-engine instruction pacing benches: NI dependency-free
    instructions per iteration on one engine (same-engine WAW only)."""
    NI = 128
    width = 2048 if "2048" in mode else (1024 if "1024" in mode else 512)
    accum = "na" not in mode
    edt = mybir.dt.float8e4 if "f8" in mode else (
        mybir.dt.float16 if "h" in mode.split("1024")[-1] + mode.split("512")[-1] else BF16)
    accum_psum = mode.endswith("p")
    with tile.TileContext(nc) as tc:
        with (
            tc.tile_pool(name="consts", bufs=1) as consts,
            tc.tile_pool(name="epool", bufs=4) as epool,
            tc.tile_pool(name="ps", bufs=4, space="PSUM") as ps,
            tc.tile_pool(name="psc", bufs=1, space="PSUM") as psc,
        ):
            g_all = consts.tile([128, 4096], BF16)
            den_buf = consts.tile([128, NBLK], F32)
            num_buf = consts.tile([128, NBLK], F32)
            scr = consts.tile([128, width], edt)
            e_src = consts.tile([128, width], edt)
            warm = consts.tile([128, 1], F32)
            nc.vector.memset(warm, 0.0)
            nc.scalar.activation(warm, warm,
                                 mybir.ActivationFunctionType.Exp)
            nc.vector.memset(g_all, 0.01)
            nc.vector.memset(e_src, 1.0)
            nc.vector.memset(den_buf, 1.0)
            nc.vector.memset(num_buf, 1.0)
            if accum_psum:
                den_ps = psc.tile([128, NBLK], F32)
            l_ps = psc.tile([128, width], F32)
            for h in range(0, width, 512):
                nc.tensor.matmul(l_ps[:, h:h + 512], lhsT=g_all[:, 0:128],
                                 rhs=g_all[:, 128 + h:640 + h],
                                 start=True, stop=True)

            def body():
                for k in range(NI):
                    if mode.startswith("actb"):
                        e_t = epool.tile([128, width], edt, tag="e")
                        tgt = den_ps if accum_psum else den_buf
                        nc.scalar.activation(
                            e_t, l_ps, mybir.ActivationFunctionType.Exp,
                            accum_out=(tgt[:, k % NBLK:k % NBLK + 1]
                                       if accum else None))
                    elif mode.startswith("dveb"):
                        if accum_psum:
                            nc.vector.scalar_tensor_tensor(
                                scr, e_src, 0.0, g_all[:, 128:128 + width],
                                op0=mybir.AluOpType.bypass,
                                op1=mybir.AluOpType.mult,
                                accum_out=den_ps[:, k % NBLK:k % NBLK + 1])
                        elif "m" in mode.split("b")[1]:
                            nc.vector.scalar_tensor_tensor(
                                scr, e_src, 1.0, g_all[:, 128:128 + width],
                                op0=mybir.AluOpType.mult,
                                op1=mybir.AluOpType.mult,
                                accum_out=num_buf[:, k % NBLK:k % NBLK + 1])
                        elif "t" in mode.split("b")[1]:
                            nc.vector.tensor_tensor(
                                scr, e_src, g_all[:, 128:128 + width],
                                op=mybir.AluOpType.mult)
                        else:
                            nc.vector.scalar_tensor_tensor(
                                scr, e_src, 0.0, g_all[:, 128:128 + width],
                                op0=mybir.AluOpType.bypass,
                                op1=mybir.AluOpType.mult,
                                accum_out=num_buf[:, k % NBLK:k % NBLK + 1])
                    elif mode.startswith("redb"):
                        nc.vector.tensor_reduce(
                            num_buf[:, k % NBLK:k % NBLK + 1], e_src,
                            axis=mybir.AxisListType.X,
                            op=mybir.AluOpType.add)
                    elif mode.startswith("ttb"):
                        nc.vector.tensor_tensor(
                            scr, e_src, g_all[:, 128:128 + width],
                            op=mybir.AluOpType.mult)
                    elif mode.startswith("ttrb"):
                        nc.vector.tensor_tensor_reduce(
                            scr, e_src, g_all[:, 128:128 + width],
                            1.0, 0.0,
                            op0=mybir.AluOpType.mult,
                            op1=mybir.AluOpType.add,
                            accum_out=num_buf[:, k % NBLK:k % NBLK + 1])
                    else:  # mmb
                        o = ps.tile([128, 512], F32, tag="l")
                        nc.tensor.matmul(o, lhsT=g_all[:, 0:128],
                                         rhs=g_all[:, 128:640],
                                         start=True, stop=True)
                nc.sync.dma_start(out_d[:, 0:NPAIR], den_buf[:, 0:NPAIR])
                nc.sync.dma_start(out_d[:, NPAIR:2 * NPAIR],
                                  num_buf[:, 0:NPAIR])

            if loop_reps > 1:
                with tc.For_i(0, loop_reps, 1,
                              hint_engines=(mybir.EngineType.PE,
                                            mybir.EngineType.Activation,
                                            mybir.EngineType.DVE,
                                            mybir.EngineType.SP)):
                    body()
            else:
                body()
    nc.compile()
    return nc


def _get_program(nblk=NBLK, loop_reps=1, mode="full"):
    key = ("nc", nblk, loop_reps, mode, _LAYOUT["padw"],
           _LAYOUT.get("s120", False))
    if key not in _PROGRAM_CACHE:
        _PROGRAM_CACHE[key] = _build_program(nblk, loop_reps, mode)
    return _PROGRAM_CACHE[key]


def _host_prep(pairwise_g, coset_functions, mask, w_y, w_g):
    """Build the per-core bf16 input blobs with a mask-compacted key axis.

    The free (key) axis is the flat (s2, j) list of mask-valid columns per
    batch, padded to a static width padw; pad columns carry L = -1e30 so
    their exp is exactly 0.  Order is irrelevant (den/num are plain sums).
    """
    y = np.asarray(coset_functions, dtype=np.float32)    # (B, N, S, C)
    maskb = np.asarray(mask)
    w_y0 = np.asarray(w_y, dtype=np.float32)[:, 0]
    w_g = np.asarray(w_g, dtype=np.float32)

    # valid flat key indices v = s2*N + j, per batch
    mflat = maskb.transpose(0, 2, 1).reshape(B, S * N)
    counts = mflat.sum(axis=1)
    padw = max(32, int(-(-int(counts.max()) // 32) * 32))
    str0 = YT0 + B * padw
    totw = str0 + NPAIR * padw
    _LAYOUT.update(padw=padw, str0=str0, totw=totw)

    gidx = np.zeros((B, padw), np.int64)
    pad = np.ones((B, padw), bool)
    for b in range(B):
        ix = np.flatnonzero(mflat[b])
        gidx[b, :len(ix)] = ix
        pad[b, :len(ix)] = False

    # ycols[b, v, c] = y[b, j(v), s2(v), c]
    yv = y.transpose(0, 2, 1, 3).reshape(B, S * N, CIN)
    ycols = np.stack([yv[b, gidx[b]] for b in range(B)])  # (B, padw, C)
    # L rows (C, padw): w_y0*y on valid cols, -1e30 on pads
    ld = w_y0 * ycols
    ld[pad] = -1e30
    ld = np.ascontiguousarray(ld.transpose(0, 2, 1))      # (B, C, padw)
    # y table, zeroed on pads, duplicated over (i2, s1)
    ytab = np.where(pad[..., None], 0.0, ycols).transpose(0, 2, 1)
    yt = np.broadcast_to(ytab[:, None], (B, 16, CIN, padw))
    yt_plane = yt.reshape(B, 128, padw).transpose(1, 0, 2).reshape(128, -1)

    # stationary lhsT (128, 128): out col k = i2*64 + s1*8 + c
    lhsT = np.zeros((128, 128), np.float32)
    for i2 in range(2):
        for s1 in range(S):
            for g in range(GDIM):
                p = i2 * 56 + s1 * GDIM + g
                k0 = i2 * 64 + s1 * 8
                lhsT[p, k0:k0 + CIN] = w_g[:, g]
    for c in range(CIN):
        for i2 in range(2):
            for s1 in range(S):
                lhsT[112 + c, i2 * 64 + s1 * 8 + c] = 1.0

    consts_plane = np.empty((128, str0), NPBF16)
    consts_plane[:, LHS0:LHS0 + 128] = lhsT
    consts_plane[:, YT0:YT0 + B * padw] = yt_plane

    pairwise_g = np.asarray(pairwise_g, dtype=np.float32)
    in_maps = []
    for k in range(NCORES):
        sl = slice(ISHARD * k, ISHARD * (k + 1))
        pg = pairwise_g[:, sl]                   # (B, 16, j, s1, s2, g)
        x = pg.reshape(B, 8, 2, N, S, S, GDIM)
        x = x.transpose(0, 1, 2, 4, 6, 5, 3)     # [b,ip,i2,s1,g,s2,j]
        x = np.ascontiguousarray(x).reshape(B, 8, 2, S, GDIM, S * N)
        xg = np.stack([x[b][..., gidx[b]] for b in range(B)])
        xg = xg.reshape(NPAIR, 112, padw)
        if _LAYOUT.get("s120", False):
            strm = np.empty((CROWS, NPAIR, padw), NPBF16)
            strm[0:112] = xg.transpose(1, 0, 2)
            strm[112:CROWS] = np.repeat(ld, NPAIR // B, axis=0) \
                .transpose(1, 0, 2)
            in_maps.append({"blob": np.ascontiguousarray(consts_plane),
                            "strm": strm.reshape(CROWS, -1)})
        else:
            blob = np.empty((128, totw), NPBF16)
            blob[:, :str0] = consts_plane
            stream = blob[:, str0:].reshape(128, NPAIR, padw)
            stream[0:112] = xg.transpose(1, 0, 2)
            stream[112:CROWS] = np.repeat(ld, NPAIR // B, axis=0) \
                .transpose(1, 0, 2)
            in_maps.append({"blob": blob})
    return in_maps


def _host_finish(s_list, coset_functions, mask, w_lin):
    """Decode per-core (128, 64) outputs into the full result."""
    y = np.asarray(coset_functions, dtype=np.float32)
    maskf = np.asarray(mask).astype(np.float32)
    w_lin = np.asarray(w_lin, dtype=np.float32)
    out = np.empty((B, N, S, COUT), np.float32)
    for k in range(NCORES):
        s = np.asarray(s_list[k], dtype=np.float32)      # (128, 2*NPAIR)
        # partition p = i2*64 + s1*8 + c; block = 2*pp + i2
        den = s[:, :NPAIR].reshape(2, S, CIN, NPAIR).transpose(3, 0, 1, 2)
        num = s[:, NPAIR:].reshape(2, S, CIN, NPAIR).transpose(3, 0, 1, 2)
        den = den.reshape(NBLK, S, CIN)
        num = num.reshape(NBLK, S, CIN)
        sl = slice(ISHARD * k, ISHARD * (k + 1))
        y_q = y[:, sl].reshape(NBLK, S, CIN)
        m_q = maskf[:, sl].reshape(NBLK, S)
        res = (y_q + num / den) * m_q[..., None]
        res = res @ w_lin.T
        out[:, sl] = res.reshape(B, ISHARD, S, COUT)
    return out


def kernel(pairwise_g, coset_functions, mask, w_y, b_y, w_g, b_g, w_lin):
    pairwise_g = np.asarray(pairwise_g)
    coset_functions = np.asarray(coset_functions)
    mask = np.asarray(mask)

    in_maps = _host_prep(pairwise_g, coset_functions, mask,
                         np.asarray(w_y), np.asarray(w_g))
    nc = _get_program()
    res = run_bass_kernel_spmd(nc, in_maps, core_ids=list(range(NCORES)))
    s_list = [r["out_s"] for r in res.results]
    return _host_finish(s_list, coset_functions, mask, np.asarray(w_lin))
